# revision 9
# baseline (speedup 1.0000x reference)
"""Trainium2 Bass kernel for nn_Attention_43516608643501.

Cross-attention: Q = out_d [T,B,H]; K = V = sum of fwd/bwd halves of out_e
-> [S,B,H]; scores = Q @ K^T per batch (contraction over H, no scaling);
softmax over the source dim S; context = P @ V -> output [T,B,H].

Sharding: data-parallel over batch (dim 1): 2 batches per core x 8 cores,
no cross-core communication.

v3 design: one flattened software pipeline over 32 (batch, t-tile)
iterations.  Scores are computed in [t_partition, s_free] tiles so the
softmax max/sum are free-dim reductions (DVE reduce_max + ACT exp
accum_out).  P is transposed back to [s,t] on the PE (fp16, 1 cyc/row).
Per iteration g the PE queue is:
  [P-transposes(g-1)] [input-prep transposes] [MM1(g)] [MM2(g-1)]
so the PSUM->SBUF copies of P^T(g-1) (DVE) hide under MM1(g), and the
input-prep transposes' DMA+add dependencies were issued >=1 iteration
earlier.  exp(g) is queued on ACT before out-scale(g-1) so the strict
ACT FIFO can't delay MM1(g+1)'s PSUM-bank reuse.  Batch 1's loads run
during batch 0's tiles 6..13 and its transposes during tiles 8..15, so
the batch boundary has no pipeline bubble.

Numerics: both matmuls run in fp16 (full PE rate; fp16's 4.9e-4 rounding
vs bf16's 4e-3 matters because the scores carry no 1/sqrt(H) scaling, so
near-ties in the softmax amplify score error by exp()).
"""

import numpy as np
from contextlib import ExitStack

S, T, B, H = 2048, 2048, 16, 512
NCORES = 8
BLOC = B // NCORES  # batches per core
P128 = 128
NS = S // P128  # 16 s-tiles
NT = T // P128  # 16 t-tiles
NH = H // P128  # 4 h-chunks of the contraction
SC = 512  # s-chunk width (scores tile columns)
NSC = S // SC  # 4 s-chunks per t-tile

_cached_nc = None


def _build():
    import concourse.bacc as bacc
    import concourse.tile as tile
    from concourse import mybir
    from concourse.masks import make_identity

    f32 = mybir.dt.float32
    f16 = mybir.dt.float16

    nc = bacc.Bacc(None, target_bir_lowering=False)
    d_oe = nc.dram_tensor("out_e", [S, BLOC, 2 * H], f32, kind="ExternalInput")
    d_od = nc.dram_tensor("out_d", [T, BLOC, H], f32, kind="ExternalInput")
    d_out = nc.dram_tensor("out", [T, BLOC, H], f32, kind="ExternalOutput")

    with ExitStack() as ctx:
        tc = ctx.enter_context(tile.TileContext(nc))
        singles = ctx.enter_context(tc.tile_pool(name="singles", bufs=1))
        loads = ctx.enter_context(tc.tile_pool(name="loads", bufs=6))
        persist = ctx.enter_context(tc.tile_pool(name="persist", bufs=2))
        work = ctx.enter_context(tc.tile_pool(name="work", bufs=4))
        ptile = ctx.enter_context(tc.tile_pool(name="ptile", bufs=2))
        outs = ctx.enter_context(tc.tile_pool(name="outs", bufs=3))
        small = ctx.enter_context(tc.tile_pool(name="small", bufs=3))
        # PSUM: 8 banks = ps_s0..3 (4) + tr (2) + ps_c (2)
        ps_s_pool = ctx.enter_context(tc.tile_pool(name="ps_s_pool", bufs=1, space="PSUM"))
        ps_tr = ctx.enter_context(tc.tile_pool(name="ps_tr", bufs=2, space="PSUM"))
        ps_cp = ctx.enter_context(tc.tile_pool(name="ps_cp", bufs=2, space="PSUM"))

        id16 = singles.tile([P128, P128], f16)
        make_identity(nc, id16)

        # per-batch persistent tile handles (persist pool tags rotate
        # bufs=2 slots, so consecutive batches double-buffer)
        st = [dict(oeT=[None] * NSC, odT=[None] * NT, nat=[None] * NS,
                   odf=[None] * NT) for _ in range(BLOC)]

        raws = {}

        def oe_dma(b, k):
            raw = loads.tile([P128, 2 * H], f32, tag="raw", name="raw")
            nc.sync.dma_start(out=raw, in_=d_oe[k * P128:(k + 1) * P128, b, :])
            raws[(b, k)] = raw

        def oe_add(b, k):
            raw = raws.pop((b, k))
            nat = persist.tile([P128, H], f16, tag=f"oenat{k}", name=f"oenat{k}")
            nc.vector.tensor_add(nat, raw[:, 0:H], raw[:, H:2 * H])
            st[b]["nat"][k] = nat

        def oe_load(b, k):
            oe_dma(b, k)
            oe_add(b, k)

        def oe_tr(b, k):
            nat = st[b]["nat"][k]
            ci, j = k // 4, k % 4
            if st[b]["oeT"][ci] is None or j == 0:
                st[b]["oeT"][ci] = persist.tile(
                    [P128, NH, SC], f16, tag=f"oeT{ci}", name=f"oeT{ci}"
                )
            trp = ps_tr.tile([P128, H], f16, tag="tr", name="tr_oe")
            for hc in range(NH):
                nc.tensor.transpose(
                    trp[:, hc * P128:(hc + 1) * P128],
                    nat[:, hc * P128:(hc + 1) * P128],
                    id16,
                )
            dst = st[b]["oeT"][ci][:, :, j * P128:(j + 1) * P128]
            src = trp.rearrange("p (h s) -> p h s", h=NH)
            nc.scalar.copy(dst, src)

        def od_load(b, tt):
            odr = loads.tile([P128, H], f32, tag="odr", name="odr")
            nc.sync.dma_start(out=odr, in_=d_od[tt * P128:(tt + 1) * P128, b, :])
            odf = work.tile([P128, H], f16, tag="odf", name="odf")
            nc.vector.tensor_copy(odf, odr)
            st[b]["odf"][tt] = odf

        def od_tr(b, tt):
            odf = st[b]["odf"][tt]
            trp = ps_tr.tile([P128, H], f16, tag="tr", name="tr_od")
            for hc in range(NH):
                nc.tensor.transpose(
                    trp[:, hc * P128:(hc + 1) * P128],
                    odf[:, hc * P128:(hc + 1) * P128],
                    id16,
                )
            odT = persist.tile([P128, NH, P128], f16, tag=f"odT{tt}", name=f"odT{tt}")
            nc.scalar.copy(odT, trp.rearrange("p (h t) -> p h t", h=NH))
            st[b]["odT"][tt] = odT

        def ptr_stage(prev):
            """PE transposes of P(g-1) [t,s]->[s,t] + DVE PSUM->SBUF copies."""
            _, _, pts, _ = prev
            pTs = []
            for half in range(2):
                ptr = ps_tr.tile([P128, 2, SC], f16, tag="tr", name="ptr")
                for sub in range(2):
                    ci = 2 * half + sub
                    for j in range(SC // P128):
                        nc.tensor.transpose(
                            ptr[:, sub, j * P128:(j + 1) * P128],
                            pts[ci][:, j * P128:(j + 1) * P128],
                            id16,
                        )
                pT = ptile.tile([P128, 2 * SC], f16, tag=f"pT{half}", name=f"pT{half}")
                nc.vector.tensor_copy(pT, ptr.rearrange("p a b -> p (a b)"))
                pTs.append(pT)
            return pTs

        def mm1_chunk(b, tt, ci, mx, pss_list):
            pss = ps_s_pool.tile([P128, SC], f32, tag=f"ps_s{ci}", name=f"ps_s{ci}")
            odT = st[b]["odT"][tt]
            oeT = st[b]["oeT"][ci]
            for hc in range(NH):
                nc.tensor.matmul(
                    pss,
                    odT[:, hc, :],
                    oeT[:, hc, :],
                    start=(hc == 0),
                    stop=(hc == NH - 1),
                )
            nc.vector.reduce_max(mx[:, ci:ci + 1], pss, axis=mybir.AxisListType.X)
            pss_list.append(pss)

        def mm1(b, tt):
            """hc-outer ordering: 4 consecutive MMs share the stationary
            odT[:, hc, :], accumulating into the 4 score banks in parallel."""
            mx = small.tile([P128, NSC], f32, tag="mx", name="mx")
            pss_list = [
                ps_s_pool.tile([P128, SC], f32, tag=f"ps_s{ci}", name=f"ps_s{ci}")
                for ci in range(NSC)
            ]
            odT = st[b]["odT"][tt]
            for hc in range(NH):
                for ci in range(NSC):
                    nc.tensor.matmul(
                        pss_list[ci],
                        odT[:, hc, :],
                        st[b]["oeT"][ci][:, hc, :],
                        start=(hc == 0),
                        stop=(hc == NH - 1),
                        skip_group_check=True,
                    )
            for ci in range(NSC):
                nc.vector.reduce_max(
                    mx[:, ci:ci + 1], pss_list[ci], axis=mybir.AxisListType.X
                )
            return mx, pss_list

        def softmax_exp(b, tt, mx, pss_list):
            m = small.tile([P128, 1], f32, tag="m", name="m")
            nc.vector.reduce_max(m, mx, axis=mybir.AxisListType.X)
            neg_m = small.tile([P128, 1], f32, tag="neg_m", name="neg_m")
            nc.vector.tensor_scalar_mul(neg_m, m, -1.0)
            lacc = small.tile([P128, NSC], f32, tag="lacc", name="lacc")
            pts = []
            for ci in range(NSC):
                pt = ptile.tile([P128, SC], f16, tag=f"pt{ci}", name=f"pt{ci}")
                nc.scalar.activation(
                    pt, pss_list[ci],
                    mybir.ActivationFunctionType.Exp,
                    bias=neg_m, scale=1.0,
                    accum_out=lacc[:, ci:ci + 1],
                )
                pts.append(pt)
            l = small.tile([P128, 1], f32, tag="l", name="l")
            nc.vector.reduce_sum(l, lacc, axis=mybir.AxisListType.X)
            linv = small.tile([P128, 1], f32, tag="linv", name="linv")
            nc.vector.reciprocal(linv, l)
            return b, tt, pts, linv

        def mm2(prev, pTs):
            pb, ptt, _, plinv = prev
            ps_c = ps_cp.tile([P128, H], f32, tag="ps_c", name="ps_c")
            nat = st[pb]["nat"]
            for k in range(NS):
                nc.tensor.matmul(
                    ps_c,
                    pTs[k // 8][:, (k % 8) * P128:(k % 8 + 1) * P128],
                    nat[k],
                    start=(k == 0), stop=(k == NS - 1),
                )
            ot = outs.tile([P128, H], f32, tag="ot", name="ot")
            nc.scalar.activation(
                ot, ps_c, mybir.ActivationFunctionType.Identity,
                bias=0.0, scale=plinv,
            )
            nc.sync.dma_start(
                out=d_out[ptt * P128:(ptt + 1) * P128, pb, :], in_=ot
            )

        # ---- intro: batch 0 tile 0, chunk-interleaved with the oe loads
        # (DMA-bound; the PE stalls here are unavoidable) ----
        od_load(0, 0)
        mx0 = small.tile([P128, NSC], f32, tag="mx", name="mx")
        pss0 = []
        for ci in range(NSC):
            for k in range(4 * ci, 4 * ci + 4):
                oe_load(0, k)
                oe_tr(0, k)
            if ci == 0:
                od_tr(0, 0)
                od_load(0, 1)
            mm1_chunk(0, 0, ci, mx0, pss0)
        od_load(0, 2)
        od_tr(0, 1)
        prev = softmax_exp(0, 0, mx0, pss0)

        # ---- steady state ----
        # next-batch oe pipeline: dma at iteration END of tt=4..11, DVE
        # adds at tt=5..12 (just after the P^T copies in the DVE FIFO, deps
        # already on-chip), PE transposes at tt=7..14.
        for g in range(1, BLOC * NT):
            b, tt = divmod(g, NT)
            # PE front: P^T(g-1), then prep transposes whose deps are ready
            pTs = ptr_stage(prev)
            if b + 1 < BLOC and 5 <= tt < 13:
                oe_add(b + 1, 2 * (tt - 5))
                oe_add(b + 1, 2 * (tt - 5) + 1)
            if tt + 1 < NT:
                od_tr(b, tt + 1)
            elif b + 1 < BLOC:
                od_tr(b + 1, 0)
            if b + 1 < BLOC and 7 <= tt < 15:
                oe_tr(b + 1, 2 * (tt - 7))
                oe_tr(b + 1, 2 * (tt - 7) + 1)
            mx, pss_list = mm1(b, tt)
            cur = softmax_exp(b, tt, mx, pss_list)  # ACT: exps before outscale
            mm2(prev, pTs)
            prev = cur
            # iteration tail: DMA issues for future iterations
            if tt + 2 < NT:
                od_load(b, tt + 2)
            elif b + 1 < BLOC and tt + 2 - NT < 2:
                od_load(b + 1, tt + 2 - NT)
            if b + 1 < BLOC and 4 <= tt < 12:
                oe_dma(b + 1, 2 * (tt - 4))
                oe_dma(b + 1, 2 * (tt - 4) + 1)
        pTs = ptr_stage(prev)
        mm2(prev, pTs)

    nc.finalize()
    return nc


def _ensure_devices():
    """Make sure the 8 NeuronCores are visible to jax.devices().

    The calling harness may have pinned jax to cpu (JAX_PLATFORMS=cpu is a
    common pin for running the jax reference); the Bass SPMD launcher uses
    jax.devices(), so re-point jax at the neuron platform if needed.
    """
    import os
    import jax

    try:
        devs = jax.devices()
    except Exception:
        devs = []
    if sum(1 for d in devs if d.platform != "cpu") >= NCORES:
        return
    for plats in ("axon,cpu", None):
        try:
            if plats is None:
                os.environ.pop("JAX_PLATFORMS", None)
            else:
                os.environ["JAX_PLATFORMS"] = plats
            jax.config.update("jax_platforms", plats)
            from jax.extend.backend import clear_backends

            clear_backends()
            devs = jax.devices()
            if sum(1 for d in devs if d.platform != "cpu") >= NCORES:
                return
        except Exception:
            continue


def kernel(in_e=None, out_e=None, out_d=None, **kwargs):
    global _cached_nc
    from concourse.bass_utils import run_bass_kernel_spmd

    _ensure_devices()

    out_e = np.asarray(out_e, dtype=np.float32)
    out_d = np.asarray(out_d, dtype=np.float32)
    if _cached_nc is None:
        _cached_nc = _build()
    in_maps = []
    for c in range(NCORES):
        bsl = slice(c * BLOC, (c + 1) * BLOC)
        in_maps.append({
            "out_e": np.ascontiguousarray(out_e[:, bsl, :]),
            "out_d": np.ascontiguousarray(out_d[:, bsl, :]),
        })
    res = run_bass_kernel_spmd(_cached_nc, in_maps, list(range(NCORES)))
    return np.concatenate([res.results[c]["out"] for c in range(NCORES)], axis=1)


# revision 11
# speedup vs baseline: 1.0016x; 1.0016x over previous
"""Trainium2 Bass kernel for nn_Attention_43516608643501.

Cross-attention: Q = out_d [T,B,H]; K = V = sum of fwd/bwd halves of out_e
-> [S,B,H]; scores = Q @ K^T per batch (contraction over H, no scaling);
softmax over the source dim S; context = P @ V -> output [T,B,H].

Sharding: data-parallel over batch (dim 1): 2 batches per core x 8 cores,
no cross-core communication.

v3 design: one flattened software pipeline over 32 (batch, t-tile)
iterations.  Scores are computed in [t_partition, s_free] tiles so the
softmax max/sum are free-dim reductions (DVE reduce_max + ACT exp
accum_out).  P is transposed back to [s,t] on the PE (fp16, 1 cyc/row).
Per iteration g the PE queue is:
  [P-transposes(g-1)] [input-prep transposes] [MM1(g)] [MM2(g-1)]
so the PSUM->SBUF copies of P^T(g-1) (DVE) hide under MM1(g), and the
input-prep transposes' DMA+add dependencies were issued >=1 iteration
earlier.  exp(g) is queued on ACT before out-scale(g-1) so the strict
ACT FIFO can't delay MM1(g+1)'s PSUM-bank reuse.  Batch 1's loads run
during batch 0's tiles 6..13 and its transposes during tiles 8..15, so
the batch boundary has no pipeline bubble.

Numerics: both matmuls run in fp16 (full PE rate; fp16's 4.9e-4 rounding
vs bf16's 4e-3 matters because the scores carry no 1/sqrt(H) scaling, so
near-ties in the softmax amplify score error by exp()).
"""

import numpy as np
from contextlib import ExitStack

S, T, B, H = 2048, 2048, 16, 512
NCORES = 8
BLOC = B // NCORES  # batches per core
P128 = 128
NS = S // P128  # 16 s-tiles
NT = T // P128  # 16 t-tiles
NH = H // P128  # 4 h-chunks of the contraction
SC = 512  # s-chunk width (scores tile columns)
NSC = S // SC  # 4 s-chunks per t-tile

_cached_nc = None


def _build():
    import concourse.bacc as bacc
    import concourse.tile as tile
    from concourse import mybir
    from concourse.masks import make_identity

    f32 = mybir.dt.float32
    f16 = mybir.dt.float16

    nc = bacc.Bacc(None, target_bir_lowering=False)
    d_oe = nc.dram_tensor("out_e", [S, BLOC, 2 * H], f32, kind="ExternalInput")
    d_od = nc.dram_tensor("out_d", [T, BLOC, H], f32, kind="ExternalInput")
    d_out = nc.dram_tensor("out", [T, BLOC, H], f32, kind="ExternalOutput")

    with ExitStack() as ctx:
        tc = ctx.enter_context(tile.TileContext(nc))
        singles = ctx.enter_context(tc.tile_pool(name="singles", bufs=1))
        loads = ctx.enter_context(tc.tile_pool(name="loads", bufs=6))
        persist = ctx.enter_context(tc.tile_pool(name="persist", bufs=2))
        work = ctx.enter_context(tc.tile_pool(name="work", bufs=4))
        ptile = ctx.enter_context(tc.tile_pool(name="ptile", bufs=2))
        outs = ctx.enter_context(tc.tile_pool(name="outs", bufs=3))
        small = ctx.enter_context(tc.tile_pool(name="small", bufs=3))
        # PSUM: 8 banks = ps_s0..3 (4) + tr (2) + ps_c (2)
        ps_s_pool = ctx.enter_context(tc.tile_pool(name="ps_s_pool", bufs=1, space="PSUM"))
        ps_tr = ctx.enter_context(tc.tile_pool(name="ps_tr", bufs=2, space="PSUM"))
        ps_cp = ctx.enter_context(tc.tile_pool(name="ps_cp", bufs=2, space="PSUM"))

        id16 = singles.tile([P128, P128], f16)
        make_identity(nc, id16)

        # per-batch persistent tile handles (persist pool tags rotate
        # bufs=2 slots, so consecutive batches double-buffer)
        st = [dict(oeT=[None] * NSC, odT=[None] * NT, nat=[None] * NS,
                   odf=[None] * NT) for _ in range(BLOC)]

        raws = {}

        def oe_dma(b, k):
            raw = loads.tile([P128, 2 * H], f32, tag="raw", name="raw")
            nc.sync.dma_start(out=raw, in_=d_oe[k * P128:(k + 1) * P128, b, :])
            raws[(b, k)] = raw

        def oe_add(b, k):
            raw = raws.pop((b, k))
            nat = persist.tile([P128, H], f16, tag=f"oenat{k}", name=f"oenat{k}")
            nc.vector.tensor_add(nat, raw[:, 0:H], raw[:, H:2 * H])
            st[b]["nat"][k] = nat

        def oe_load(b, k):
            oe_dma(b, k)
            oe_add(b, k)

        def oe_tr(b, k):
            nat = st[b]["nat"][k]
            ci, j = k // 4, k % 4
            if st[b]["oeT"][ci] is None or j == 0:
                st[b]["oeT"][ci] = persist.tile(
                    [P128, NH, SC], f16, tag=f"oeT{ci}", name=f"oeT{ci}"
                )
            trp = ps_tr.tile([P128, H], f16, tag="tr", name="tr_oe")
            for hc in range(NH):
                nc.tensor.transpose(
                    trp[:, hc * P128:(hc + 1) * P128],
                    nat[:, hc * P128:(hc + 1) * P128],
                    id16,
                )
            dst = st[b]["oeT"][ci][:, :, j * P128:(j + 1) * P128]
            src = trp.rearrange("p (h s) -> p h s", h=NH)
            nc.scalar.copy(dst, src)

        def od_load(b, tt):
            odr = loads.tile([P128, H], f32, tag="odr", name="odr")
            nc.sync.dma_start(out=odr, in_=d_od[tt * P128:(tt + 1) * P128, b, :])
            odf = work.tile([P128, H], f16, tag="odf", name="odf")
            nc.vector.tensor_copy(odf, odr)
            st[b]["odf"][tt] = odf

        def od_tr(b, tt):
            odf = st[b]["odf"][tt]
            trp = ps_tr.tile([P128, H], f16, tag="tr", name="tr_od")
            for hc in range(NH):
                nc.tensor.transpose(
                    trp[:, hc * P128:(hc + 1) * P128],
                    odf[:, hc * P128:(hc + 1) * P128],
                    id16,
                )
            odT = persist.tile([P128, NH, P128], f16, tag=f"odT{tt}", name=f"odT{tt}")
            nc.scalar.copy(odT, trp.rearrange("p (h t) -> p h t", h=NH))
            st[b]["odT"][tt] = odT

        def ptr_stage(prev):
            """PE transposes of P(g-1) [t,s]->[s,t] + DVE PSUM->SBUF copies."""
            _, _, pts, _ = prev
            pTs = []
            for half in range(2):
                ptr = ps_tr.tile([P128, 2, SC], f16, tag="tr", name="ptr")
                for sub in range(2):
                    ci = 2 * half + sub
                    for j in range(SC // P128):
                        nc.tensor.transpose(
                            ptr[:, sub, j * P128:(j + 1) * P128],
                            pts[ci][:, j * P128:(j + 1) * P128],
                            id16,
                        )
                pT = ptile.tile([P128, 2 * SC], f16, tag=f"pT{half}", name=f"pT{half}")
                nc.vector.tensor_copy(pT, ptr.rearrange("p a b -> p (a b)"))
                pTs.append(pT)
            return pTs

        def mm1_chunk(b, tt, ci, mx, pss_list):
            pss = ps_s_pool.tile([P128, SC], f32, tag=f"ps_s{ci}", name=f"ps_s{ci}")
            odT = st[b]["odT"][tt]
            oeT = st[b]["oeT"][ci]
            for hc in range(NH):
                nc.tensor.matmul(
                    pss,
                    odT[:, hc, :],
                    oeT[:, hc, :],
                    start=(hc == 0),
                    stop=(hc == NH - 1),
                )
            nc.vector.reduce_max(mx[:, ci:ci + 1], pss, axis=mybir.AxisListType.X)
            pss_list.append(pss)

        def mm1(b, tt):
            mx = small.tile([P128, NSC], f32, tag="mx", name="mx")
            pss_list = []
            for ci in range(NSC):
                mm1_chunk(b, tt, ci, mx, pss_list)
            return mx, pss_list

        def softmax_exp(b, tt, mx, pss_list):
            m = small.tile([P128, 1], f32, tag="m", name="m")
            nc.vector.reduce_max(m, mx, axis=mybir.AxisListType.X)
            neg_m = small.tile([P128, 1], f32, tag="neg_m", name="neg_m")
            nc.vector.tensor_scalar_mul(neg_m, m, -1.0)
            lacc = small.tile([P128, NSC], f32, tag="lacc", name="lacc")
            pts = []
            for ci in range(NSC):
                pt = ptile.tile([P128, SC], f16, tag=f"pt{ci}", name=f"pt{ci}")
                nc.scalar.activation(
                    pt, pss_list[ci],
                    mybir.ActivationFunctionType.Exp,
                    bias=neg_m, scale=1.0,
                    accum_out=lacc[:, ci:ci + 1],
                )
                pts.append(pt)
            l = small.tile([P128, 1], f32, tag="l", name="l")
            nc.vector.reduce_sum(l, lacc, axis=mybir.AxisListType.X)
            linv = small.tile([P128, 1], f32, tag="linv", name="linv")
            nc.vector.reciprocal(linv, l)
            return b, tt, pts, linv

        def mm2(prev, pTs):
            pb, ptt, _, plinv = prev
            ps_c = ps_cp.tile([P128, H], f32, tag="ps_c", name="ps_c")
            nat = st[pb]["nat"]
            for k in range(NS):
                nc.tensor.matmul(
                    ps_c,
                    pTs[k // 8][:, (k % 8) * P128:(k % 8 + 1) * P128],
                    nat[k],
                    start=(k == 0), stop=(k == NS - 1),
                )
            ot = outs.tile([P128, H], f32, tag="ot", name="ot")
            nc.scalar.activation(
                ot, ps_c, mybir.ActivationFunctionType.Identity,
                bias=0.0, scale=plinv,
            )
            nc.sync.dma_start(
                out=d_out[ptt * P128:(ptt + 1) * P128, pb, :], in_=ot
            )

        # ---- intro: batch 0 tile 0, sub-chunk (2 oe tiles -> N=256 MMs)
        # interleaved with the oe loads so the PE starts as soon as the
        # first two s-tiles land (DMA-paced region) ----
        od_load(0, 0)
        mx0 = small.tile([P128, NSC], f32, tag="mx", name="mx")
        pss0 = []
        HC2 = SC // 2
        for sub in range(2 * NSC):
            ci, j = divmod(sub, 2)
            for k in range(4 * ci + 2 * j, 4 * ci + 2 * j + 2):
                oe_load(0, k)
                oe_tr(0, k)
            if sub == 0:
                od_tr(0, 0)
                od_load(0, 1)
            if j == 0:
                pss0.append(ps_s_pool.tile(
                    [P128, SC], f32, tag=f"ps_s{ci}", name=f"ps_s{ci}"
                ))
            for hc in range(NH):
                nc.tensor.matmul(
                    pss0[ci][:, j * HC2:(j + 1) * HC2],
                    st[0]["odT"][0][:, hc, :],
                    st[0]["oeT"][ci][:, hc, j * HC2:(j + 1) * HC2],
                    start=(hc == 0),
                    stop=(hc == NH - 1),
                    skip_group_check=True,
                )
            if j == 1:
                nc.vector.reduce_max(
                    mx0[:, ci:ci + 1], pss0[ci], axis=mybir.AxisListType.X
                )
        od_load(0, 2)
        od_tr(0, 1)
        prev = softmax_exp(0, 0, mx0, pss0)

        # ---- steady state ----
        # next-batch oe pipeline: dma at iteration END of tt=4..11, DVE
        # adds at tt=5..12 (just after the P^T copies in the DVE FIFO, deps
        # already on-chip), PE transposes at tt=7..14.
        for g in range(1, BLOC * NT):
            b, tt = divmod(g, NT)
            # PE front: P^T(g-1), then prep transposes whose deps are ready
            pTs = ptr_stage(prev)
            if b + 1 < BLOC and 5 <= tt < 13:
                oe_add(b + 1, 2 * (tt - 5))
                oe_add(b + 1, 2 * (tt - 5) + 1)
            if tt + 1 < NT:
                od_tr(b, tt + 1)
            elif b + 1 < BLOC:
                od_tr(b + 1, 0)
            if b + 1 < BLOC and 7 <= tt < 15:
                oe_tr(b + 1, 2 * (tt - 7))
                oe_tr(b + 1, 2 * (tt - 7) + 1)
            mx, pss_list = mm1(b, tt)
            cur = softmax_exp(b, tt, mx, pss_list)  # ACT: exps before outscale
            mm2(prev, pTs)
            prev = cur
            # iteration tail: DMA issues for future iterations
            if tt + 2 < NT:
                od_load(b, tt + 2)
            elif b + 1 < BLOC and tt + 2 - NT < 2:
                od_load(b + 1, tt + 2 - NT)
            if b + 1 < BLOC and 4 <= tt < 12:
                oe_dma(b + 1, 2 * (tt - 4))
                oe_dma(b + 1, 2 * (tt - 4) + 1)
        pTs = ptr_stage(prev)
        mm2(prev, pTs)

    nc.finalize()
    return nc


def _ensure_devices():
    """Make sure the 8 NeuronCores are visible to jax.devices().

    The calling harness may have pinned jax to cpu (JAX_PLATFORMS=cpu is a
    common pin for running the jax reference); the Bass SPMD launcher uses
    jax.devices(), so re-point jax at the neuron platform if needed.
    """
    import os
    import jax

    try:
        devs = jax.devices()
    except Exception:
        devs = []
    if sum(1 for d in devs if d.platform != "cpu") >= NCORES:
        return
    for plats in ("axon,cpu", None):
        try:
            if plats is None:
                os.environ.pop("JAX_PLATFORMS", None)
            else:
                os.environ["JAX_PLATFORMS"] = plats
            jax.config.update("jax_platforms", plats)
            from jax.extend.backend import clear_backends

            clear_backends()
            devs = jax.devices()
            if sum(1 for d in devs if d.platform != "cpu") >= NCORES:
                return
        except Exception:
            continue


def kernel(in_e=None, out_e=None, out_d=None, **kwargs):
    global _cached_nc
    from concourse.bass_utils import run_bass_kernel_spmd

    _ensure_devices()

    out_e = np.asarray(out_e, dtype=np.float32)
    out_d = np.asarray(out_d, dtype=np.float32)
    if _cached_nc is None:
        _cached_nc = _build()
    in_maps = []
    for c in range(NCORES):
        bsl = slice(c * BLOC, (c + 1) * BLOC)
        in_maps.append({
            "out_e": np.ascontiguousarray(out_e[:, bsl, :]),
            "out_d": np.ascontiguousarray(out_d[:, bsl, :]),
        })
    res = run_bass_kernel_spmd(_cached_nc, in_maps, list(range(NCORES)))
    return np.concatenate([res.results[c]["out"] for c in range(NCORES)], axis=1)


# revision 13
# speedup vs baseline: 1.0111x; 1.0095x over previous
"""Trainium2 Bass kernel for nn_Attention_43516608643501.

Cross-attention: Q = out_d [T,B,H]; K = V = sum of fwd/bwd halves of out_e
-> [S,B,H]; scores = Q @ K^T per batch (contraction over H, no scaling);
softmax over the source dim S; context = P @ V -> output [T,B,H].

Sharding: data-parallel over batch (dim 1): 2 batches per core x 8 cores,
no cross-core communication.

v3 design: one flattened software pipeline over 32 (batch, t-tile)
iterations.  Scores are computed in [t_partition, s_free] tiles so the
softmax max/sum are free-dim reductions (DVE reduce_max + ACT exp
accum_out).  P is transposed back to [s,t] on the PE (fp16, 1 cyc/row).
Per iteration g the PE queue is:
  [P-transposes(g-1)] [input-prep transposes] [MM1(g)] [MM2(g-1)]
so the PSUM->SBUF copies of P^T(g-1) (DVE) hide under MM1(g), and the
input-prep transposes' DMA+add dependencies were issued >=1 iteration
earlier.  exp(g) is queued on ACT before out-scale(g-1) so the strict
ACT FIFO can't delay MM1(g+1)'s PSUM-bank reuse.  Batch 1's loads run
during batch 0's tiles 6..13 and its transposes during tiles 8..15, so
the batch boundary has no pipeline bubble.

Numerics: both matmuls run in fp16 (full PE rate; fp16's 4.9e-4 rounding
vs bf16's 4e-3 matters because the scores carry no 1/sqrt(H) scaling, so
near-ties in the softmax amplify score error by exp()).
"""

import numpy as np
from contextlib import ExitStack

S, T, B, H = 2048, 2048, 16, 512
NCORES = 8
BLOC = B // NCORES  # batches per core
P128 = 128
NS = S // P128  # 16 s-tiles
NT = T // P128  # 16 t-tiles
NH = H // P128  # 4 h-chunks of the contraction
SC = 512  # s-chunk width (scores tile columns)
NSC = S // SC  # 4 s-chunks per t-tile

_cached_nc = None


def _build():
    import concourse.bacc as bacc
    import concourse.tile as tile
    from concourse import mybir
    from concourse.masks import make_identity

    f32 = mybir.dt.float32
    f16 = mybir.dt.float16

    nc = bacc.Bacc(None, target_bir_lowering=False)
    d_oe = nc.dram_tensor("out_e", [S, BLOC, 2 * H], f32, kind="ExternalInput")
    d_od = nc.dram_tensor("out_d", [T, BLOC, H], f32, kind="ExternalInput")
    d_out = nc.dram_tensor("out", [T, BLOC, H], f32, kind="ExternalOutput")

    with ExitStack() as ctx:
        tc = ctx.enter_context(tile.TileContext(nc))
        singles = ctx.enter_context(tc.tile_pool(name="singles", bufs=1))
        loads = ctx.enter_context(tc.tile_pool(name="loads", bufs=8))
        persist = ctx.enter_context(tc.tile_pool(name="persist", bufs=2))
        work = ctx.enter_context(tc.tile_pool(name="work", bufs=4))
        ptile = ctx.enter_context(tc.tile_pool(name="ptile", bufs=2))
        outs = ctx.enter_context(tc.tile_pool(name="outs", bufs=3))
        small = ctx.enter_context(tc.tile_pool(name="small", bufs=3))
        # PSUM: 8 banks = ps_s0..3 (4) + tr (2) + ps_c (2)
        ps_s_pool = ctx.enter_context(tc.tile_pool(name="ps_s_pool", bufs=1, space="PSUM"))
        ps_tr = ctx.enter_context(tc.tile_pool(name="ps_tr", bufs=2, space="PSUM"))
        ps_cp = ctx.enter_context(tc.tile_pool(name="ps_cp", bufs=2, space="PSUM"))

        id16 = singles.tile([P128, P128], f16)
        make_identity(nc, id16)

        # per-batch persistent tile handles (persist pool tags rotate
        # bufs=2 slots, so consecutive batches double-buffer)
        st = [dict(oeT=[None] * NSC, odT=[None] * NT, nat=[None] * NS,
                   odf=[None] * NT) for _ in range(BLOC)]

        raws = {}

        def oe_dma(b, k):
            raw = loads.tile([P128, 2 * H], f32, tag="raw", name="raw")
            nc.sync.dma_start(out=raw, in_=d_oe[k * P128:(k + 1) * P128, b, :])
            raws[(b, k)] = raw

        def oe_add(b, k):
            raw = raws.pop((b, k))
            nat = persist.tile([P128, H], f16, tag=f"oenat{k}", name=f"oenat{k}")
            nc.vector.tensor_add(nat, raw[:, 0:H], raw[:, H:2 * H])
            st[b]["nat"][k] = nat

        def oe_load(b, k):
            oe_dma(b, k)
            oe_add(b, k)

        def oe_tr(b, k):
            nat = st[b]["nat"][k]
            ci, j = k // 4, k % 4
            if st[b]["oeT"][ci] is None or j == 0:
                st[b]["oeT"][ci] = persist.tile(
                    [P128, NH, SC], f16, tag=f"oeT{ci}", name=f"oeT{ci}"
                )
            trp = ps_tr.tile([P128, H], f16, tag="tr", name="tr_oe")
            for hc in range(NH):
                nc.tensor.transpose(
                    trp[:, hc * P128:(hc + 1) * P128],
                    nat[:, hc * P128:(hc + 1) * P128],
                    id16,
                )
            dst = st[b]["oeT"][ci][:, :, j * P128:(j + 1) * P128]
            src = trp.rearrange("p (h s) -> p h s", h=NH)
            nc.scalar.copy(dst, src)

        def od_load(b, tt):
            odr = loads.tile([P128, H], f32, tag="odr", name="odr")
            nc.sync.dma_start(out=odr, in_=d_od[tt * P128:(tt + 1) * P128, b, :])
            odf = work.tile([P128, H], f16, tag="odf", name="odf")
            nc.vector.tensor_copy(odf, odr)
            st[b]["odf"][tt] = odf

        def od_tr(b, tt):
            odf = st[b]["odf"][tt]
            trp = ps_tr.tile([P128, H], f16, tag="tr", name="tr_od")
            for hc in range(NH):
                nc.tensor.transpose(
                    trp[:, hc * P128:(hc + 1) * P128],
                    odf[:, hc * P128:(hc + 1) * P128],
                    id16,
                )
            odT = persist.tile([P128, NH, P128], f16, tag=f"odT{tt}", name=f"odT{tt}")
            nc.scalar.copy(odT, trp.rearrange("p (h t) -> p h t", h=NH))
            st[b]["odT"][tt] = odT

        def ptr_stage(prev):
            """PE transposes of P(g-1) [t,s]->[s,t] + DVE PSUM->SBUF copies."""
            _, _, pts, _ = prev
            pTs = []
            for half in range(2):
                ptr = ps_tr.tile([P128, 2, SC], f16, tag="tr", name="ptr")
                for sub in range(2):
                    ci = 2 * half + sub
                    for j in range(SC // P128):
                        nc.tensor.transpose(
                            ptr[:, sub, j * P128:(j + 1) * P128],
                            pts[ci][:, j * P128:(j + 1) * P128],
                            id16,
                        )
                pT = ptile.tile([P128, 2 * SC], f16, tag=f"pT{half}", name=f"pT{half}")
                nc.vector.tensor_copy(pT, ptr.rearrange("p a b -> p (a b)"))
                pTs.append(pT)
            return pTs

        def mm1_chunk(b, tt, ci, mx, pss_list):
            pss = ps_s_pool.tile([P128, SC], f32, tag=f"ps_s{ci}", name=f"ps_s{ci}")
            odT = st[b]["odT"][tt]
            oeT = st[b]["oeT"][ci]
            for hc in range(NH):
                nc.tensor.matmul(
                    pss,
                    odT[:, hc, :],
                    oeT[:, hc, :],
                    start=(hc == 0),
                    stop=(hc == NH - 1),
                )
            nc.vector.reduce_max(mx[:, ci:ci + 1], pss, axis=mybir.AxisListType.X)
            pss_list.append(pss)

        def mm1(b, tt):
            mx = small.tile([P128, NSC], f32, tag="mx", name="mx")
            pss_list = []
            for ci in range(NSC):
                mm1_chunk(b, tt, ci, mx, pss_list)
            return mx, pss_list

        def softmax_exp(b, tt, mx, pss_list):
            m = small.tile([P128, 1], f32, tag="m", name="m")
            nc.vector.reduce_max(m, mx, axis=mybir.AxisListType.X)
            neg_m = small.tile([P128, 1], f32, tag="neg_m", name="neg_m")
            nc.vector.tensor_scalar_mul(neg_m, m, -1.0)
            lacc = small.tile([P128, NSC], f32, tag="lacc", name="lacc")
            pts = []
            for ci in range(NSC):
                pt = ptile.tile([P128, SC], f16, tag=f"pt{ci}", name=f"pt{ci}")
                nc.scalar.activation(
                    pt, pss_list[ci],
                    mybir.ActivationFunctionType.Exp,
                    bias=neg_m, scale=1.0,
                    accum_out=lacc[:, ci:ci + 1],
                )
                pts.append(pt)
            l = small.tile([P128, 1], f32, tag="l", name="l")
            nc.vector.reduce_sum(l, lacc, axis=mybir.AxisListType.X)
            linv = small.tile([P128, 1], f32, tag="linv", name="linv")
            nc.vector.reciprocal(linv, l)
            return b, tt, pts, linv

        def mm2(prev, pTs):
            pb, ptt, _, plinv = prev
            ps_c = ps_cp.tile([P128, H], f32, tag="ps_c", name="ps_c")
            nat = st[pb]["nat"]
            for k in range(NS):
                nc.tensor.matmul(
                    ps_c,
                    pTs[k // 8][:, (k % 8) * P128:(k % 8 + 1) * P128],
                    nat[k],
                    start=(k == 0), stop=(k == NS - 1),
                )
            ot = outs.tile([P128, H], f32, tag="ot", name="ot")
            nc.scalar.activation(
                ot, ps_c, mybir.ActivationFunctionType.Identity,
                bias=0.0, scale=plinv,
            )
            nc.sync.dma_start(
                out=d_out[ptt * P128:(ptt + 1) * P128, pb, :], in_=ot
            )

        # ---- intro: batch 0 tile 0, chunk-interleaved with the oe loads
        # (DMA-bound; the PE stalls here are unavoidable) ----
        od_load(0, 0)
        mx0 = small.tile([P128, NSC], f32, tag="mx", name="mx")
        pss0 = []
        for ci in range(NSC):
            for k in range(4 * ci, 4 * ci + 4):
                oe_load(0, k)
                oe_tr(0, k)
            if ci == 0:
                od_tr(0, 0)
                od_load(0, 1)
            mm1_chunk(0, 0, ci, mx0, pss0)
        od_load(0, 2)
        od_tr(0, 1)
        prev = softmax_exp(0, 0, mx0, pss0)

        # ---- steady state ----
        # next-batch oe pipeline: dma at iteration END of tt=4..11, DVE
        # adds at tt=5..12 (just after the P^T copies in the DVE FIFO, deps
        # already on-chip), PE transposes at tt=7..14.
        for g in range(1, BLOC * NT):
            b, tt = divmod(g, NT)
            # PE front: P^T(g-1), then prep transposes whose deps are ready
            pTs = ptr_stage(prev)
            if b + 1 < BLOC and 5 <= tt < 13:
                oe_add(b + 1, 2 * (tt - 5))
                oe_add(b + 1, 2 * (tt - 5) + 1)
            if tt + 1 < NT:
                od_tr(b, tt + 1)
            elif b + 1 < BLOC:
                od_tr(b + 1, 0)
            if b + 1 < BLOC and 7 <= tt < 15:
                oe_tr(b + 1, 2 * (tt - 7))
                oe_tr(b + 1, 2 * (tt - 7) + 1)
            mx, pss_list = mm1(b, tt)
            cur = softmax_exp(b, tt, mx, pss_list)  # ACT: exps before outscale
            mm2(prev, pTs)
            prev = cur
            # iteration tail: DMA issues for future iterations
            if tt + 2 < NT:
                od_load(b, tt + 2)
            elif b + 1 < BLOC and tt + 2 - NT < 2:
                od_load(b + 1, tt + 2 - NT)
            if b + 1 < BLOC and 3 <= tt < 11:
                oe_dma(b + 1, 2 * (tt - 3))
                oe_dma(b + 1, 2 * (tt - 3) + 1)
        pTs = ptr_stage(prev)
        mm2(prev, pTs)

    nc.finalize()
    return nc


def _ensure_devices():
    """Make sure the 8 NeuronCores are visible to jax.devices().

    The calling harness may have pinned jax to cpu (JAX_PLATFORMS=cpu is a
    common pin for running the jax reference); the Bass SPMD launcher uses
    jax.devices(), so re-point jax at the neuron platform if needed.
    """
    import os
    import jax

    try:
        devs = jax.devices()
    except Exception:
        devs = []
    if sum(1 for d in devs if d.platform != "cpu") >= NCORES:
        return
    for plats in ("axon,cpu", None):
        try:
            if plats is None:
                os.environ.pop("JAX_PLATFORMS", None)
            else:
                os.environ["JAX_PLATFORMS"] = plats
            jax.config.update("jax_platforms", plats)
            from jax.extend.backend import clear_backends

            clear_backends()
            devs = jax.devices()
            if sum(1 for d in devs if d.platform != "cpu") >= NCORES:
                return
        except Exception:
            continue


def kernel(in_e=None, out_e=None, out_d=None, **kwargs):
    global _cached_nc
    from concourse.bass_utils import run_bass_kernel_spmd

    _ensure_devices()

    out_e = np.asarray(out_e, dtype=np.float32)
    out_d = np.asarray(out_d, dtype=np.float32)
    if _cached_nc is None:
        _cached_nc = _build()
    in_maps = []
    for c in range(NCORES):
        bsl = slice(c * BLOC, (c + 1) * BLOC)
        in_maps.append({
            "out_e": np.ascontiguousarray(out_e[:, bsl, :]),
            "out_d": np.ascontiguousarray(out_d[:, bsl, :]),
        })
    res = run_bass_kernel_spmd(_cached_nc, in_maps, list(range(NCORES)))
    return np.concatenate([res.results[c]["out"] for c in range(NCORES)], axis=1)


# revision 14
# speedup vs baseline: 1.0164x; 1.0053x over previous
"""Trainium2 Bass kernel for nn_Attention_43516608643501.

Cross-attention: Q = out_d [T,B,H]; K = V = sum of fwd/bwd halves of out_e
-> [S,B,H]; scores = Q @ K^T per batch (contraction over H, no scaling);
softmax over the source dim S; context = P @ V -> output [T,B,H].

Sharding: data-parallel over batch (dim 1): 2 batches per core x 8 cores,
no cross-core communication.

v3 design: one flattened software pipeline over 32 (batch, t-tile)
iterations.  Scores are computed in [t_partition, s_free] tiles so the
softmax max/sum are free-dim reductions (DVE reduce_max + ACT exp
accum_out).  P is transposed back to [s,t] on the PE (fp16, 1 cyc/row).
Per iteration g the PE queue is:
  [P-transposes(g-1)] [input-prep transposes] [MM1(g)] [MM2(g-1)]
so the PSUM->SBUF copies of P^T(g-1) (DVE) hide under MM1(g), and the
input-prep transposes' DMA+add dependencies were issued >=1 iteration
earlier.  exp(g) is queued on ACT before out-scale(g-1) so the strict
ACT FIFO can't delay MM1(g+1)'s PSUM-bank reuse.  Batch 1's loads run
during batch 0's tiles 6..13 and its transposes during tiles 8..15, so
the batch boundary has no pipeline bubble.

Numerics: both matmuls run in fp16 (full PE rate; fp16's 4.9e-4 rounding
vs bf16's 4e-3 matters because the scores carry no 1/sqrt(H) scaling, so
near-ties in the softmax amplify score error by exp()).
"""

import numpy as np
from contextlib import ExitStack

S, T, B, H = 2048, 2048, 16, 512
NCORES = 8
BLOC = B // NCORES  # batches per core
P128 = 128
NS = S // P128  # 16 s-tiles
NT = T // P128  # 16 t-tiles
NH = H // P128  # 4 h-chunks of the contraction
SC = 512  # s-chunk width (scores tile columns)
NSC = S // SC  # 4 s-chunks per t-tile

_cached_nc = None


def _build():
    import concourse.bacc as bacc
    import concourse.tile as tile
    from concourse import mybir
    from concourse.masks import make_identity

    f32 = mybir.dt.float32
    f16 = mybir.dt.float16

    nc = bacc.Bacc(None, target_bir_lowering=False)
    d_oe = nc.dram_tensor("out_e", [S, BLOC, 2 * H], f32, kind="ExternalInput")
    d_od = nc.dram_tensor("out_d", [T, BLOC, H], f32, kind="ExternalInput")
    d_out = nc.dram_tensor("out", [T, BLOC, H], f32, kind="ExternalOutput")

    with ExitStack() as ctx:
        tc = ctx.enter_context(tile.TileContext(nc))
        singles = ctx.enter_context(tc.tile_pool(name="singles", bufs=1))
        loads = ctx.enter_context(tc.tile_pool(name="loads", bufs=8))
        persist = ctx.enter_context(tc.tile_pool(name="persist", bufs=2))
        work = ctx.enter_context(tc.tile_pool(name="work", bufs=4))
        ptile = ctx.enter_context(tc.tile_pool(name="ptile", bufs=2))
        outs = ctx.enter_context(tc.tile_pool(name="outs", bufs=3))
        small = ctx.enter_context(tc.tile_pool(name="small", bufs=3))
        # PSUM: 8 banks = ps_s0..3 (4) + tr (2) + ps_c (2)
        ps_s_pool = ctx.enter_context(tc.tile_pool(name="ps_s_pool", bufs=1, space="PSUM"))
        ps_tr = ctx.enter_context(tc.tile_pool(name="ps_tr", bufs=2, space="PSUM"))
        ps_cp = ctx.enter_context(tc.tile_pool(name="ps_cp", bufs=2, space="PSUM"))

        id16 = singles.tile([P128, P128], f16)
        make_identity(nc, id16)

        # per-batch persistent tile handles (persist pool tags rotate
        # bufs=2 slots, so consecutive batches double-buffer)
        st = [dict(oeT=[None] * NSC, odT=[None] * NT, nat=[None] * NS,
                   odf=[None] * NT) for _ in range(BLOC)]

        raws = {}

        def oe_dma(b, k):
            raw = loads.tile([P128, 2 * H], f32, tag="raw", name="raw")
            nc.sync.dma_start(out=raw, in_=d_oe[k * P128:(k + 1) * P128, b, :])
            raws[(b, k)] = raw

        def oe_add(b, k):
            raw = raws.pop((b, k))
            nat = persist.tile([P128, H], f16, tag=f"oenat{k}", name=f"oenat{k}")
            nc.gpsimd.tensor_add(nat, raw[:, 0:H], raw[:, H:2 * H])
            st[b]["nat"][k] = nat

        def oe_load(b, k):
            oe_dma(b, k)
            oe_add(b, k)

        def oe_tr(b, k):
            nat = st[b]["nat"][k]
            ci, j = k // 4, k % 4
            if st[b]["oeT"][ci] is None or j == 0:
                st[b]["oeT"][ci] = persist.tile(
                    [P128, NH, SC], f16, tag=f"oeT{ci}", name=f"oeT{ci}"
                )
            trp = ps_tr.tile([P128, H], f16, tag="tr", name="tr_oe")
            for hc in range(NH):
                nc.tensor.transpose(
                    trp[:, hc * P128:(hc + 1) * P128],
                    nat[:, hc * P128:(hc + 1) * P128],
                    id16,
                )
            dst = st[b]["oeT"][ci][:, :, j * P128:(j + 1) * P128]
            src = trp.rearrange("p (h s) -> p h s", h=NH)
            nc.scalar.copy(dst, src)

        def od_load(b, tt):
            odr = loads.tile([P128, H], f32, tag="odr", name="odr")
            nc.sync.dma_start(out=odr, in_=d_od[tt * P128:(tt + 1) * P128, b, :])
            odf = work.tile([P128, H], f16, tag="odf", name="odf")
            nc.gpsimd.tensor_copy(odf, odr)
            st[b]["odf"][tt] = odf

        def od_tr(b, tt):
            odf = st[b]["odf"][tt]
            trp = ps_tr.tile([P128, H], f16, tag="tr", name="tr_od")
            for hc in range(NH):
                nc.tensor.transpose(
                    trp[:, hc * P128:(hc + 1) * P128],
                    odf[:, hc * P128:(hc + 1) * P128],
                    id16,
                )
            odT = persist.tile([P128, NH, P128], f16, tag=f"odT{tt}", name=f"odT{tt}")
            nc.scalar.copy(odT, trp.rearrange("p (h t) -> p h t", h=NH))
            st[b]["odT"][tt] = odT

        def ptr_stage(prev):
            """PE transposes of P(g-1) [t,s]->[s,t] + DVE PSUM->SBUF copies."""
            _, _, pts, _ = prev
            pTs = []
            for half in range(2):
                ptr = ps_tr.tile([P128, 2, SC], f16, tag="tr", name="ptr")
                for sub in range(2):
                    ci = 2 * half + sub
                    for j in range(SC // P128):
                        nc.tensor.transpose(
                            ptr[:, sub, j * P128:(j + 1) * P128],
                            pts[ci][:, j * P128:(j + 1) * P128],
                            id16,
                        )
                pT = ptile.tile([P128, 2 * SC], f16, tag=f"pT{half}", name=f"pT{half}")
                nc.vector.tensor_copy(pT, ptr.rearrange("p a b -> p (a b)"))
                pTs.append(pT)
            return pTs

        def mm1_chunk(b, tt, ci, mx, pss_list):
            pss = ps_s_pool.tile([P128, SC], f32, tag=f"ps_s{ci}", name=f"ps_s{ci}")
            odT = st[b]["odT"][tt]
            oeT = st[b]["oeT"][ci]
            for hc in range(NH):
                nc.tensor.matmul(
                    pss,
                    odT[:, hc, :],
                    oeT[:, hc, :],
                    start=(hc == 0),
                    stop=(hc == NH - 1),
                )
            nc.vector.reduce_max(mx[:, ci:ci + 1], pss, axis=mybir.AxisListType.X)
            pss_list.append(pss)

        def mm1(b, tt):
            mx = small.tile([P128, NSC], f32, tag="mx", name="mx")
            pss_list = []
            for ci in range(NSC):
                mm1_chunk(b, tt, ci, mx, pss_list)
            return mx, pss_list

        def softmax_exp(b, tt, mx, pss_list):
            m = small.tile([P128, 1], f32, tag="m", name="m")
            nc.vector.reduce_max(m, mx, axis=mybir.AxisListType.X)
            neg_m = small.tile([P128, 1], f32, tag="neg_m", name="neg_m")
            nc.vector.tensor_scalar_mul(neg_m, m, -1.0)
            lacc = small.tile([P128, NSC], f32, tag="lacc", name="lacc")
            pts = []
            for ci in range(NSC):
                pt = ptile.tile([P128, SC], f16, tag=f"pt{ci}", name=f"pt{ci}")
                nc.scalar.activation(
                    pt, pss_list[ci],
                    mybir.ActivationFunctionType.Exp,
                    bias=neg_m, scale=1.0,
                    accum_out=lacc[:, ci:ci + 1],
                )
                pts.append(pt)
            l = small.tile([P128, 1], f32, tag="l", name="l")
            nc.vector.reduce_sum(l, lacc, axis=mybir.AxisListType.X)
            linv = small.tile([P128, 1], f32, tag="linv", name="linv")
            nc.vector.reciprocal(linv, l)
            return b, tt, pts, linv

        def mm2(prev, pTs):
            pb, ptt, _, plinv = prev
            ps_c = ps_cp.tile([P128, H], f32, tag="ps_c", name="ps_c")
            nat = st[pb]["nat"]
            for k in range(NS):
                nc.tensor.matmul(
                    ps_c,
                    pTs[k // 8][:, (k % 8) * P128:(k % 8 + 1) * P128],
                    nat[k],
                    start=(k == 0), stop=(k == NS - 1),
                )
            ot = outs.tile([P128, H], f32, tag="ot", name="ot")
            nc.scalar.activation(
                ot, ps_c, mybir.ActivationFunctionType.Identity,
                bias=0.0, scale=plinv,
            )
            nc.sync.dma_start(
                out=d_out[ptt * P128:(ptt + 1) * P128, pb, :], in_=ot
            )

        # ---- intro: batch 0 tile 0, chunk-interleaved with the oe loads
        # (DMA-bound; the PE stalls here are unavoidable) ----
        od_load(0, 0)
        mx0 = small.tile([P128, NSC], f32, tag="mx", name="mx")
        pss0 = []
        for ci in range(NSC):
            for k in range(4 * ci, 4 * ci + 4):
                oe_load(0, k)
                oe_tr(0, k)
            if ci == 0:
                od_tr(0, 0)
                od_load(0, 1)
            mm1_chunk(0, 0, ci, mx0, pss0)
        od_load(0, 2)
        od_tr(0, 1)
        prev = softmax_exp(0, 0, mx0, pss0)

        # ---- steady state ----
        # next-batch oe pipeline: dma at iteration END of tt=4..11, DVE
        # adds at tt=5..12 (just after the P^T copies in the DVE FIFO, deps
        # already on-chip), PE transposes at tt=7..14.
        for g in range(1, BLOC * NT):
            b, tt = divmod(g, NT)
            # PE front: P^T(g-1), then prep transposes whose deps are ready
            pTs = ptr_stage(prev)
            if b + 1 < BLOC and 5 <= tt < 13:
                oe_add(b + 1, 2 * (tt - 5))
                oe_add(b + 1, 2 * (tt - 5) + 1)
            if tt + 1 < NT:
                od_tr(b, tt + 1)
            elif b + 1 < BLOC:
                od_tr(b + 1, 0)
            if b + 1 < BLOC and 7 <= tt < 15:
                oe_tr(b + 1, 2 * (tt - 7))
                oe_tr(b + 1, 2 * (tt - 7) + 1)
            mx, pss_list = mm1(b, tt)
            cur = softmax_exp(b, tt, mx, pss_list)  # ACT: exps before outscale
            mm2(prev, pTs)
            prev = cur
            # iteration tail: DMA issues for future iterations
            if tt + 2 < NT:
                od_load(b, tt + 2)
            elif b + 1 < BLOC and tt + 2 - NT < 2:
                od_load(b + 1, tt + 2 - NT)
            if b + 1 < BLOC and 3 <= tt < 11:
                oe_dma(b + 1, 2 * (tt - 3))
                oe_dma(b + 1, 2 * (tt - 3) + 1)
        pTs = ptr_stage(prev)
        mm2(prev, pTs)

    nc.finalize()
    return nc


def _ensure_devices():
    """Make sure the 8 NeuronCores are visible to jax.devices().

    The calling harness may have pinned jax to cpu (JAX_PLATFORMS=cpu is a
    common pin for running the jax reference); the Bass SPMD launcher uses
    jax.devices(), so re-point jax at the neuron platform if needed.
    """
    import os
    import jax

    try:
        devs = jax.devices()
    except Exception:
        devs = []
    if sum(1 for d in devs if d.platform != "cpu") >= NCORES:
        return
    for plats in ("axon,cpu", None):
        try:
            if plats is None:
                os.environ.pop("JAX_PLATFORMS", None)
            else:
                os.environ["JAX_PLATFORMS"] = plats
            jax.config.update("jax_platforms", plats)
            from jax.extend.backend import clear_backends

            clear_backends()
            devs = jax.devices()
            if sum(1 for d in devs if d.platform != "cpu") >= NCORES:
                return
        except Exception:
            continue


def kernel(in_e=None, out_e=None, out_d=None, **kwargs):
    global _cached_nc
    from concourse.bass_utils import run_bass_kernel_spmd

    _ensure_devices()

    out_e = np.asarray(out_e, dtype=np.float32)
    out_d = np.asarray(out_d, dtype=np.float32)
    if _cached_nc is None:
        _cached_nc = _build()
    in_maps = []
    for c in range(NCORES):
        bsl = slice(c * BLOC, (c + 1) * BLOC)
        in_maps.append({
            "out_e": np.ascontiguousarray(out_e[:, bsl, :]),
            "out_d": np.ascontiguousarray(out_d[:, bsl, :]),
        })
    res = run_bass_kernel_spmd(_cached_nc, in_maps, list(range(NCORES)))
    return np.concatenate([res.results[c]["out"] for c in range(NCORES)], axis=1)


# revision 15
# speedup vs baseline: 1.0243x; 1.0077x over previous
"""Trainium2 Bass kernel for nn_Attention_43516608643501.

Cross-attention: Q = out_d [T,B,H]; K = V = sum of fwd/bwd halves of out_e
-> [S,B,H]; scores = Q @ K^T per batch (contraction over H, no scaling);
softmax over the source dim S; context = P @ V -> output [T,B,H].

Sharding: data-parallel over batch (dim 1): 2 batches per core x 8 cores,
no cross-core communication.

v3 design: one flattened software pipeline over 32 (batch, t-tile)
iterations.  Scores are computed in [t_partition, s_free] tiles so the
softmax max/sum are free-dim reductions (DVE reduce_max + ACT exp
accum_out).  P is transposed back to [s,t] on the PE (fp16, 1 cyc/row).
Per iteration g the PE queue is:
  [P-transposes(g-1)] [input-prep transposes] [MM1(g)] [MM2(g-1)]
so the PSUM->SBUF copies of P^T(g-1) (DVE) hide under MM1(g), and the
input-prep transposes' DMA+add dependencies were issued >=1 iteration
earlier.  exp(g) is queued on ACT before out-scale(g-1) so the strict
ACT FIFO can't delay MM1(g+1)'s PSUM-bank reuse.  Batch 1's loads run
during batch 0's tiles 6..13 and its transposes during tiles 8..15, so
the batch boundary has no pipeline bubble.

Numerics: both matmuls run in fp16 (full PE rate; fp16's 4.9e-4 rounding
vs bf16's 4e-3 matters because the scores carry no 1/sqrt(H) scaling, so
near-ties in the softmax amplify score error by exp()).
"""

import numpy as np
from contextlib import ExitStack

S, T, B, H = 2048, 2048, 16, 512
NCORES = 8
BLOC = B // NCORES  # batches per core
P128 = 128
NS = S // P128  # 16 s-tiles
NT = T // P128  # 16 t-tiles
NH = H // P128  # 4 h-chunks of the contraction
SC = 512  # s-chunk width (scores tile columns)
NSC = S // SC  # 4 s-chunks per t-tile

_cached_nc = None


def _build():
    import concourse.bacc as bacc
    import concourse.tile as tile
    from concourse import mybir
    from concourse.masks import make_identity

    f32 = mybir.dt.float32
    f16 = mybir.dt.float16

    nc = bacc.Bacc(None, target_bir_lowering=False)
    d_oe = nc.dram_tensor("out_e", [S, BLOC, 2 * H], f32, kind="ExternalInput")
    d_od = nc.dram_tensor("out_d", [T, BLOC, H], f32, kind="ExternalInput")
    d_out = nc.dram_tensor("out", [T, BLOC, H], f32, kind="ExternalOutput")

    with ExitStack() as ctx:
        tc = ctx.enter_context(tile.TileContext(nc))
        singles = ctx.enter_context(tc.tile_pool(name="singles", bufs=1))
        loads = ctx.enter_context(tc.tile_pool(name="loads", bufs=8))
        persist = ctx.enter_context(tc.tile_pool(name="persist", bufs=2))
        work = ctx.enter_context(tc.tile_pool(name="work", bufs=4))
        ptile = ctx.enter_context(tc.tile_pool(name="ptile", bufs=2))
        outs = ctx.enter_context(tc.tile_pool(name="outs", bufs=3))
        small = ctx.enter_context(tc.tile_pool(name="small", bufs=3))
        # PSUM: 8 banks = ps_s0..3 (4) + tr (2) + ps_c (2)
        ps_s_pool = ctx.enter_context(tc.tile_pool(name="ps_s_pool", bufs=1, space="PSUM"))
        ps_tr = ctx.enter_context(tc.tile_pool(name="ps_tr", bufs=2, space="PSUM"))
        ps_cp = ctx.enter_context(tc.tile_pool(name="ps_cp", bufs=2, space="PSUM"))

        id16 = singles.tile([P128, P128], f16)
        make_identity(nc, id16)

        # per-batch persistent tile handles (persist pool tags rotate
        # bufs=2 slots, so consecutive batches double-buffer)
        st = [dict(oeT=[None] * NSC, odT=[None] * NT, nat=[None] * NS,
                   odf=[None] * NT) for _ in range(BLOC)]

        raws = {}

        def oe_dma(b, k):
            raw = loads.tile([P128, 2 * H], f32, tag="raw", name="raw", bufs=10)
            nc.sync.dma_start(out=raw, in_=d_oe[k * P128:(k + 1) * P128, b, :])
            raws[(b, k)] = raw

        def oe_add(b, k):
            raw = raws.pop((b, k))
            nat = persist.tile([P128, H], f16, tag=f"oenat{k}", name=f"oenat{k}")
            nc.gpsimd.tensor_add(nat, raw[:, 0:H], raw[:, H:2 * H])
            st[b]["nat"][k] = nat

        def oe_load(b, k):
            oe_dma(b, k)
            oe_add(b, k)

        def oe_tr(b, k):
            nat = st[b]["nat"][k]
            ci, j = k // 4, k % 4
            if st[b]["oeT"][ci] is None or j == 0:
                st[b]["oeT"][ci] = persist.tile(
                    [P128, NH, SC], f16, tag=f"oeT{ci}", name=f"oeT{ci}"
                )
            trp = ps_tr.tile([P128, H], f16, tag="tr", name="tr_oe")
            for hc in range(NH):
                nc.tensor.transpose(
                    trp[:, hc * P128:(hc + 1) * P128],
                    nat[:, hc * P128:(hc + 1) * P128],
                    id16,
                )
            dst = st[b]["oeT"][ci][:, :, j * P128:(j + 1) * P128]
            src = trp.rearrange("p (h s) -> p h s", h=NH)
            nc.scalar.copy(dst, src)

        def od_load(b, tt):
            odr = loads.tile([P128, H], f32, tag="odr", name="odr")
            nc.sync.dma_start(out=odr, in_=d_od[tt * P128:(tt + 1) * P128, b, :])
            odf = work.tile([P128, H], f16, tag="odf", name="odf")
            nc.gpsimd.tensor_copy(odf, odr)
            st[b]["odf"][tt] = odf

        def od_tr(b, tt):
            odf = st[b]["odf"][tt]
            trp = ps_tr.tile([P128, H], f16, tag="tr", name="tr_od")
            for hc in range(NH):
                nc.tensor.transpose(
                    trp[:, hc * P128:(hc + 1) * P128],
                    odf[:, hc * P128:(hc + 1) * P128],
                    id16,
                )
            odT = persist.tile([P128, NH, P128], f16, tag=f"odT{tt}", name=f"odT{tt}")
            nc.scalar.copy(odT, trp.rearrange("p (h t) -> p h t", h=NH))
            st[b]["odT"][tt] = odT

        def ptr_stage(prev):
            """PE transposes of P(g-1) [t,s]->[s,t] + DVE PSUM->SBUF copies."""
            _, _, pts, _ = prev
            pTs = []
            for half in range(2):
                ptr = ps_tr.tile([P128, 2, SC], f16, tag="tr", name="ptr")
                for sub in range(2):
                    ci = 2 * half + sub
                    for j in range(SC // P128):
                        nc.tensor.transpose(
                            ptr[:, sub, j * P128:(j + 1) * P128],
                            pts[ci][:, j * P128:(j + 1) * P128],
                            id16,
                        )
                pT = ptile.tile([P128, 2 * SC], f16, tag=f"pT{half}", name=f"pT{half}")
                nc.vector.tensor_copy(pT, ptr.rearrange("p a b -> p (a b)"))
                pTs.append(pT)
            return pTs

        def mm1_chunk(b, tt, ci, mx, pss_list):
            pss = ps_s_pool.tile([P128, SC], f32, tag=f"ps_s{ci}", name=f"ps_s{ci}")
            odT = st[b]["odT"][tt]
            oeT = st[b]["oeT"][ci]
            for hc in range(NH):
                nc.tensor.matmul(
                    pss,
                    odT[:, hc, :],
                    oeT[:, hc, :],
                    start=(hc == 0),
                    stop=(hc == NH - 1),
                )
            nc.vector.reduce_max(mx[:, ci:ci + 1], pss, axis=mybir.AxisListType.X)
            pss_list.append(pss)

        def mm1(b, tt):
            mx = small.tile([P128, NSC], f32, tag="mx", name="mx")
            pss_list = []
            for ci in range(NSC):
                mm1_chunk(b, tt, ci, mx, pss_list)
            return mx, pss_list

        def softmax_exp(b, tt, mx, pss_list):
            m = small.tile([P128, 1], f32, tag="m", name="m")
            nc.vector.reduce_max(m, mx, axis=mybir.AxisListType.X)
            neg_m = small.tile([P128, 1], f32, tag="neg_m", name="neg_m")
            nc.vector.tensor_scalar_mul(neg_m, m, -1.0)
            lacc = small.tile([P128, NSC], f32, tag="lacc", name="lacc")
            pts = []
            for ci in range(NSC):
                pt = ptile.tile([P128, SC], f16, tag=f"pt{ci}", name=f"pt{ci}")
                nc.scalar.activation(
                    pt, pss_list[ci],
                    mybir.ActivationFunctionType.Exp,
                    bias=neg_m, scale=1.0,
                    accum_out=lacc[:, ci:ci + 1],
                )
                pts.append(pt)
            l = small.tile([P128, 1], f32, tag="l", name="l")
            nc.vector.reduce_sum(l, lacc, axis=mybir.AxisListType.X)
            linv = small.tile([P128, 1], f32, tag="linv", name="linv")
            nc.vector.reciprocal(linv, l)
            return b, tt, pts, linv

        def mm2(prev, pTs):
            pb, ptt, _, plinv = prev
            ps_c = ps_cp.tile([P128, H], f32, tag="ps_c", name="ps_c")
            nat = st[pb]["nat"]
            for k in range(NS):
                nc.tensor.matmul(
                    ps_c,
                    pTs[k // 8][:, (k % 8) * P128:(k % 8 + 1) * P128],
                    nat[k],
                    start=(k == 0), stop=(k == NS - 1),
                )
            ot = outs.tile([P128, H], f32, tag="ot", name="ot")
            nc.scalar.activation(
                ot, ps_c, mybir.ActivationFunctionType.Identity,
                bias=0.0, scale=plinv,
            )
            nc.sync.dma_start(
                out=d_out[ptt * P128:(ptt + 1) * P128, pb, :], in_=ot
            )

        # ---- intro: batch 0 tile 0, chunk-interleaved with the oe loads
        # (DMA-bound; the PE stalls here are unavoidable) ----
        od_load(0, 0)
        mx0 = small.tile([P128, NSC], f32, tag="mx", name="mx")
        pss0 = []
        for ci in range(NSC):
            for k in range(4 * ci, 4 * ci + 4):
                oe_load(0, k)
                oe_tr(0, k)
            if ci == 0:
                od_tr(0, 0)
                od_load(0, 1)
            mm1_chunk(0, 0, ci, mx0, pss0)
        od_load(0, 2)
        od_tr(0, 1)
        prev = softmax_exp(0, 0, mx0, pss0)

        # ---- steady state ----
        # next-batch oe pipeline: dma at iteration END of tt=4..11, DVE
        # adds at tt=5..12 (just after the P^T copies in the DVE FIFO, deps
        # already on-chip), PE transposes at tt=7..14.
        for g in range(1, BLOC * NT):
            b, tt = divmod(g, NT)
            # PE front: P^T(g-1), then prep transposes whose deps are ready
            pTs = ptr_stage(prev)
            if b + 1 < BLOC and 5 <= tt < 13:
                oe_add(b + 1, 2 * (tt - 5))
                oe_add(b + 1, 2 * (tt - 5) + 1)
            if tt + 1 < NT:
                od_tr(b, tt + 1)
            elif b + 1 < BLOC:
                od_tr(b + 1, 0)
            if b + 1 < BLOC and 7 <= tt < 15:
                oe_tr(b + 1, 2 * (tt - 7))
                oe_tr(b + 1, 2 * (tt - 7) + 1)
            mx, pss_list = mm1(b, tt)
            cur = softmax_exp(b, tt, mx, pss_list)  # ACT: exps before outscale
            mm2(prev, pTs)
            prev = cur
            # iteration tail: DMA issues for future iterations
            if tt + 2 < NT:
                od_load(b, tt + 2)
            elif b + 1 < BLOC and tt + 2 - NT < 2:
                od_load(b + 1, tt + 2 - NT)
            if b + 1 < BLOC and 3 <= tt < 11:
                oe_dma(b + 1, 2 * (tt - 3))
                oe_dma(b + 1, 2 * (tt - 3) + 1)
        pTs = ptr_stage(prev)
        mm2(prev, pTs)

    nc.finalize()
    return nc


def _ensure_devices():
    """Make sure the 8 NeuronCores are visible to jax.devices().

    The calling harness may have pinned jax to cpu (JAX_PLATFORMS=cpu is a
    common pin for running the jax reference); the Bass SPMD launcher uses
    jax.devices(), so re-point jax at the neuron platform if needed.
    """
    import os
    import jax

    try:
        devs = jax.devices()
    except Exception:
        devs = []
    if sum(1 for d in devs if d.platform != "cpu") >= NCORES:
        return
    for plats in ("axon,cpu", None):
        try:
            if plats is None:
                os.environ.pop("JAX_PLATFORMS", None)
            else:
                os.environ["JAX_PLATFORMS"] = plats
            jax.config.update("jax_platforms", plats)
            from jax.extend.backend import clear_backends

            clear_backends()
            devs = jax.devices()
            if sum(1 for d in devs if d.platform != "cpu") >= NCORES:
                return
        except Exception:
            continue


def kernel(in_e=None, out_e=None, out_d=None, **kwargs):
    global _cached_nc
    from concourse.bass_utils import run_bass_kernel_spmd

    _ensure_devices()

    out_e = np.asarray(out_e, dtype=np.float32)
    out_d = np.asarray(out_d, dtype=np.float32)
    if _cached_nc is None:
        _cached_nc = _build()
    in_maps = []
    for c in range(NCORES):
        bsl = slice(c * BLOC, (c + 1) * BLOC)
        in_maps.append({
            "out_e": np.ascontiguousarray(out_e[:, bsl, :]),
            "out_d": np.ascontiguousarray(out_d[:, bsl, :]),
        })
    res = run_bass_kernel_spmd(_cached_nc, in_maps, list(range(NCORES)))
    return np.concatenate([res.results[c]["out"] for c in range(NCORES)], axis=1)
